# revision 42
# baseline (speedup 1.0000x reference)
"""Trainium2 Bass kernel for nn_Attention_850403524681.

Windowed attention block: LayerNorm -> FiLM (cond) -> QKV -> per-head
RMS-norm(q,k) -> attention with rel-pos bias -> out projection.

Full shapes: x (512, 65, 1024) f32, cond (512, 1024) f32.
Sharding: data-parallel over the 512 window-batch dim across 8 cores
(64 windows per core); all parameters replicated.

Per-core dataflow (B=64 windows, T=65 tokens, NT=4160, D=1024):
  token-major LN (stats per token) -> xn -> DRAM -> DMA-xbar-transpose
  -> d-major xnT -> FiLM applied with window-broadcast APs -> x_fT
  -> QKV matmuls (token-major out) -> RMS-norm q/k token-major
  -> DRAM -> DMA-transpose -> feature-major qT/kT (gamma folded into kT)
  -> per-window attention: simT = kT.T@qT (4x row-tiled PE), exp on ACT,
     rel-pos bias as exp(bias) multiply, attnV via PE with ones-column
     augmented V giving softmax denominators, per-partition normalize
  -> attn_out -> DRAM -> DMA-transpose -> out projection.
"""

import sys

if "/opt/trn_rl_repo" not in sys.path:
    sys.path.insert(0, "/opt/trn_rl_repo")

import numpy as np
from contextlib import ExitStack

import concourse.bass as bass
import concourse.mybir as mybir
import concourse.tile as tile
import concourse.bacc as bacc

import functools

import concourse.hw_specs as _hw_specs

# The act-table-load pass picks the FIRST table set containing each
# activation function, so programs mixing exp and ln thrash between
# 'exp_and_others' and 'natural_log' (one ~2.7us table load per op).  Steer
# both functions to the combined 'natural_log_exp_and_others' set by hiding
# them from every other set.  Set ids keep their act_info.json positions, so
# the hardware still loads the real (correct) tables.
_orig_gat = _hw_specs.get_activation_tables


@functools.cache
def _patched_gat(arch):
    out = {}
    for name, fns in _orig_gat(arch).items():
        fns = set(fns)
        if name != "natural_log_exp_and_others":
            fns.discard(mybir.ActivationFunctionType.Exp)
            fns.discard(mybir.ActivationFunctionType.Ln)
        out[name] = fns
    return out


_hw_specs.get_activation_tables = _patched_gat
bacc.get_activation_tables = _patched_gat

import ml_dtypes

BF16 = ml_dtypes.bfloat16

DIM = 1024
HEADS = 32
DIM_HEAD = 32
WINDOW = 8
NUM_REG = 1
NUM_REL = (2 * WINDOW - 1) ** 2  # 225
T = WINDOW * WINDOW + NUM_REG  # 65 tokens per window
N_CORES = 8

F32 = mybir.dt.float32
BF = mybir.dt.bfloat16
AF = mybir.ActivationFunctionType
ALU = mybir.AluOpType


def _rel_pos_indices():
    pos = np.arange(WINDOW)
    gi, gj = np.meshgrid(pos, pos, indexing="ij")
    grid = np.stack([gi, gj], axis=-1).reshape(-1, 2)
    rel = grid[:, None, :] - grid[None, :, :] + (WINDOW - 1)
    idx = rel[..., 0] * (2 * WINDOW - 1) + rel[..., 1]
    out = np.full((T, T), NUM_REL, dtype=np.int32)
    out[NUM_REG:, NUM_REG:] = idx
    return out


REL_IDX = _rel_pos_indices()

# head-within-half (hh) -> exp_sb column-block j: the sim PSUM evacuation
# AP iterates (bank, slot) bank-major while head hh sits at bank hh%4,
# slot hh//4; j(hh) is the 4x4 transpose permutation (self-inverse).
def _blk(hh):
    return (hh % 4) * 4 + hh // 4


def _bc(ap, n):
    """Append a broadcast (stride 0) innermost free dim of size n."""
    return bass.AP(ap.tensor, ap.offset, ap.ap + [[0, n]])


def build_program(B, debug_taps=False):
    """Build the per-core Bass program for B windows (B*65 % 16 == 0)."""
    NT = B * T
    assert NT % 16 == 0, "DMA transpose needs row counts divisible by 16"
    n_tok_tiles = (NT + 127) // 128

    nc = bacc.Bacc("TRN2", target_bir_lowering=False, debug=False)

    x_in = nc.dram_tensor("x", [B, T, DIM], F32, kind="ExternalInput").ap()
    condT = nc.dram_tensor("condT", [DIM, B], BF, kind="ExternalInput").ap()
    fw1 = nc.dram_tensor("fw1", [DIM, 2 * DIM], BF, kind="ExternalInput").ap()
    fb1 = nc.dram_tensor("fb1", [2 * DIM], F32, kind="ExternalInput").ap()
    fw2 = nc.dram_tensor("fw2", [2 * DIM, 2 * DIM], BF, kind="ExternalInput").ap()
    fb2 = nc.dram_tensor("fb2", [2 * DIM], F32, kind="ExternalInput").ap()
    wqk = nc.dram_tensor("wqk", [DIM, 2 * DIM], BF, kind="ExternalInput").ap()
    wv = nc.dram_tensor("wv", [DIM, DIM], BF, kind="ExternalInput").ap()
    wout = nc.dram_tensor("wout", [DIM, DIM], BF, kind="ExternalInput").ap()
    gcomb = nc.dram_tensor("gcomb", [DIM], F32, kind="ExternalInput").ap()
    expbT = nc.dram_tensor("expbT", [T, 2 * 16 * T], BF, kind="ExternalInput").ap()
    out_d = nc.dram_tensor("out", [B, T, DIM], F32, kind="ExternalOutput").ap()

    dbg = dict(kind="ExternalOutput") if debug_taps else {}
    xn_d = nc.dram_tensor("xn_d", [NT, DIM], BF, **dbg).ap()
    qk_d = nc.dram_tensor("qk_d", [NT, 2 * DIM], BF, **dbg).ap()
    v_d = nc.dram_tensor("v_d", [NT, HEADS * 33], BF, **dbg).ap()
    attn_d = nc.dram_tensor("attn_d", [NT, DIM], BF, **dbg).ap()
    gb_dbg = nc.dram_tensor("gb_dbg", [128, 16, B], BF, **dbg).ap() if debug_taps else None
    xfT_dbg = nc.dram_tensor("xfT_dbg", [8, 128, NT], BF, **dbg).ap() if debug_taps else None
    xnT_dbg = nc.dram_tensor("xnT_dbg", [8, 128, NT], BF, **dbg).ap() if debug_taps else None

    x_flat = x_in.rearrange("b t d -> (b t) d")
    out_flat = out_d.rearrange("b t d -> (b t) d")

    with tile.TileContext(nc) as tc, ExitStack() as top:
        consts = top.enter_context(tc.tile_pool(name="consts", bufs=1))

        # ---- constant loads -------------------------------------------------
        wqk_sb = consts.tile([128, 8, 2 * DIM], BF)
        nc.sync.dma_start(out=wqk_sb, in_=wqk.rearrange("(c p) j -> p c j", p=128))
        wv_sb = consts.tile([128, 8, DIM], BF)
        nc.sync.dma_start(out=wv_sb, in_=wv.rearrange("(c p) j -> p c j", p=128))
        wout_sb = consts.tile([128, 8, DIM], BF)
        nc.sync.dma_start(out=wout_sb, in_=wout.rearrange("(c p) j -> p c j", p=128))
        gcomb_sb = consts.tile([128, 8], F32)
        nc.sync.dma_start(out=gcomb_sb, in_=gcomb.rearrange("(c p) -> p c", p=128))
        expb_sb = consts.tile([128, 2 * 16 * T], BF)
        nc.sync.dma_start(out=expb_sb[:T, :], in_=expbT)
        gbT_sb = consts.tile([128, 16, B], BF)  # chunks 0-7 gammaT, 8-15 betaT
        eps_ln = consts.tile([128, 1], F32)
        nc.vector.memset(eps_ln, 1e-5)
        eps_rms = consts.tile([128, 1], F32)
        nc.vector.memset(eps_rms, 1e-24)

        # ---- phase 1: FiLM params (feature-major) ---------------------------
        with tc.tile_pool(name="film", bufs=1) as filmp, \
             tc.tile_pool(name="filmps", bufs=4, space="PSUM") as filmps:
            fw1_sb = filmp.tile([128, 8, 2 * DIM], BF)
            nc.sync.dma_start(out=fw1_sb, in_=fw1.rearrange("(c p) j -> p c j", p=128))
            fw2_sb = filmp.tile([128, 16, 2 * DIM], BF)
            nc.sync.dma_start(out=fw2_sb, in_=fw2.rearrange("(c p) j -> p c j", p=128))
            fb1_sb = filmp.tile([128, 16], F32)
            nc.sync.dma_start(out=fb1_sb, in_=fb1.rearrange("(c p) -> p c", p=128))
            fb2_sb = filmp.tile([128, 16], F32)
            nc.sync.dma_start(out=fb2_sb, in_=fb2.rearrange("(c p) -> p c", p=128))
            condT_sb = filmp.tile([128, 8, B], BF)
            nc.sync.dma_start(out=condT_sb, in_=condT.rearrange("(c p) b -> p c b", p=128))
            hT_sb = filmp.tile([128, 16, B], BF)

            for c in range(16):
                ps = filmps.tile([128, B], F32, tag="fps")
                for k in range(8):
                    nc.tensor.matmul(
                        ps, lhsT=fw1_sb[:, k, c * 128:(c + 1) * 128],
                        rhs=condT_sb[:, k, :], start=(k == 0), stop=(k == 7))
                lin = filmp.tile([128, B], F32, tag="lin", name=f"lin{c}")
                nc.scalar.activation(lin, ps, AF.Identity,
                                     bias=fb1_sb[:, c:c + 1], scale=1.0)
                sg = filmp.tile([128, B], F32, tag="sg", name=f"sg{c}")
                nc.scalar.activation(sg, ps, AF.Sigmoid,
                                     bias=fb1_sb[:, c:c + 1], scale=1.0)
                nc.vector.tensor_mul(hT_sb[:, c, :], lin, sg)
            for c in range(16):
                ps = filmps.tile([128, B], F32, tag="fps")
                for k in range(16):
                    nc.tensor.matmul(
                        ps, lhsT=fw2_sb[:, k, c * 128:(c + 1) * 128],
                        rhs=hT_sb[:, k, :], start=(k == 0), stop=(k == 15))
                nc.scalar.activation(gbT_sb[:, c, :], ps, AF.Identity,
                                     bias=fb2_sb[:, c:c + 1], scale=1.0)
            if debug_taps:
                nc.sync.dma_start(out=gb_dbg, in_=gbT_sb)

        # ---- phases 2-4: LN -> xnT -> FiLM-T -> QKV -> RMS-norm -------------
        with tc.tile_pool(name="ln", bufs=3) as lnp, \
             tc.tile_pool(name="lns", bufs=4) as lns, \
             tc.tile_pool(name="xfT", bufs=1) as xfTp, \
             tc.tile_pool(name="qkvev", bufs=2) as qkvev, \
             tc.tile_pool(name="qkvps", bufs=7, space="PSUM") as qkvps:

            # LN token-major -> xn_d
            for t in range(n_tok_tiles):
                r0, r1 = t * 128, min(t * 128 + 128, NT)
                R = r1 - r0
                x_t = lnp.tile([128, DIM], F32, tag="x")
                nc.sync.dma_start(out=x_t[:R], in_=x_flat[r0:r1])
                stats = lns.tile([128, 2, 6], F32, tag="st")
                for sg in range(2):
                    nc.vector.bn_stats(stats[:R, sg], x_t[:R, sg * 512:(sg + 1) * 512])
                mv = lns.tile([128, 2], F32, tag="mv")
                nc.vector.bn_aggr(mv[:R], stats[:R])
                rstd = lns.tile([128, 1], F32, tag="rs")
                nc.scalar.activation(rstd[:R], mv[:R, 1:2], AF.Sqrt, bias=eps_ln[:R])
                nc.vector.reciprocal(rstd[:R], rstd[:R])
                nm = lns.tile([128, 1], F32, tag="nm")
                nc.vector.tensor_scalar(nm[:R], mv[:R, 0:1], rstd[:R], -1.0,
                                        ALU.mult, ALU.mult)
                xn_t = lnp.tile([128, DIM], BF, tag="xn")
                nc.scalar.activation(xn_t[:R], x_t[:R], AF.Identity,
                                     bias=nm[:R], scale=rstd[:R])
                nc.sync.dma_start(out=xn_d[r0:r1], in_=xn_t[:R])

            # xnT via DMA transpose + FiLM in transposed domain
            x_fT = []
            for c in range(8):
                xnT_c = lnp.tile([128, NT], BF, tag="xnT")
                eng = nc.sync  # transposes must share one HWDGE ring:
                # concurrent xbar transposes on both rings corrupt each other
                eng.dma_start_transpose(out=xnT_c, in_=xn_d[:, c * 128:(c + 1) * 128])
                if debug_taps:
                    nc.sync.dma_start(out=xnT_dbg[c], in_=xnT_c)
                xf_c = xfTp.tile([128, NT], BF, tag=f"xfT{c}")
                xnv = xnT_c[:, :].rearrange("p (b t) -> p b t", b=B)
                xfv = xf_c[:, :].rearrange("p (b t) -> p b t", b=B)
                nc.vector.tensor_tensor(xfv, xnv, _bc(gbT_sb[:, c, :], T), ALU.mult)
                nc.vector.tensor_tensor(xfv, xfv, _bc(gbT_sb[:, 8 + c, :], T), ALU.add)
                if debug_taps:
                    nc.sync.dma_start(out=xfT_dbg[c], in_=xf_c)
                x_fT.append(xf_c)

            # QKV matmuls (token-major) + RMS-norm + stores
            for t in range(n_tok_tiles):
                r0, r1 = t * 128, min(t * 128 + 128, NT)
                R = r1 - r0
                ps = [qkvps.tile([128, 512], F32, tag="qp", name=f"qp{i}") for i in range(6)]
                for k in range(8):
                    lhsT = x_fT[k][:, r0:r1]
                    for n in range(6):
                        rhs = (wqk_sb[:, k, n * 512:(n + 1) * 512] if n < 4
                               else wv_sb[:, k, (n - 4) * 512:(n - 3) * 512])
                        nc.tensor.matmul(ps[n][:R], lhsT=lhsT, rhs=rhs,
                                         start=(k == 0), stop=(k == 7))
                q_t = qkvev.tile([128, 2 * DIM], BF, tag="qt")
                for n in range(4):
                    nc.scalar.copy(q_t[:R, n * 512:(n + 1) * 512], ps[n][:R])
                q2 = qkvev.tile([128, 2 * DIM], BF, tag="q2")
                nc.vector.tensor_mul(q2[:R], q_t[:R], q_t[:R])
                ss = lns.tile([128, 64], F32, tag="ss")
                nc.vector.tensor_reduce(
                    ss[:R], q2[:R, :].rearrange("p (h d) -> p h d", d=DIM_HEAD),
                    axis=mybir.AxisListType.X, op=ALU.add)
                nc.scalar.activation(ss[:R], ss[:R], AF.Sqrt, bias=eps_rms[:R])
                nc.vector.reciprocal(ss[:R], ss[:R])
                qv = q_t[:R, :].rearrange("p (h d) -> p h d", d=DIM_HEAD)
                nc.vector.tensor_tensor(qv, qv, _bc(ss[:R, :], DIM_HEAD), ALU.mult)
                nc.sync.dma_start(out=qk_d[r0:r1], in_=q_t[:R])
                v_t = qkvev.tile([128, HEADS, 33], BF, tag="vt")
                nc.vector.memset(v_t[:, :, 32:33], 1.0)
                for n in range(2):
                    nc.scalar.copy(
                        v_t[:R, n * 16:(n + 1) * 16, 0:32], ps[4 + n][:R])
                nc.sync.dma_start(out=v_d[r0:r1], in_=v_t[:R, :, :])

        # ---- phases 5-6: qT/kT + attention ----------------------------------
        with tc.tile_pool(name="qkT", bufs=1) as qkTp, \
             tc.tile_pool(name="att", bufs=2) as attp, \
             tc.tile_pool(name="atts", bufs=4) as atts, \
             tc.tile_pool(name="attps", bufs=2, space="PSUM") as attps:

            # process windows in groups so the resident qT/kT stays small;
            # group row counts must stay multiples of 16 for DMA transpose
            n_wg = 2 if (B % 32 == 0) else 1
            BG = B // n_wg
            for wg in range(n_wg):
              g0 = wg * BG * T
              qkT = []
              for c in range(16):
                qkT_c = qkTp.tile([128, BG * T], BF, tag=f"qkT{c}",
                                  name=f"qkT{wg}_{c}")
                eng = nc.sync  # transposes must share one HWDGE ring:
                # concurrent xbar transposes on both rings corrupt each other
                eng.dma_start_transpose(
                    out=qkT_c, in_=qk_d[g0:g0 + BG * T, c * 128:(c + 1) * 128])
                if c >= 8:  # kT: fold 32 * q_gamma * k_gamma per feature
                    nc.vector.tensor_scalar_mul(qkT_c, qkT_c, gcomb_sb[:, c - 8:c - 7])
                qkT.append(qkT_c)

              for bw in range(BG):
                b = wg * BG + bw
                c0, c1 = b * T, (b + 1) * T
                w0, w1 = bw * T, (bw + 1) * T
                v_b = attp.tile([128, HEADS * 33], BF, tag="vb")
                nc.sync.dma_start(out=v_b[:T], in_=v_d[c0:c1])
                for hf in range(2):
                    sim = attps.tile([128, 2048], F32, tag="ap")
                    for hh in range(16):
                        h = hf * 16 + hh
                        ch, rb = h // 4, 32 * (h % 4)
                        col = 512 * (hh % 4) + T * (hh // 4)
                        nc.tensor.matmul(
                            sim[0:T, col:col + T],
                            lhsT=qkT[8 + ch][rb:rb + 32, w0:w1],
                            rhs=qkT[ch][rb:rb + 32, w0:w1],
                            start=True, stop=True, tile_position=(rb, 0))
                    exp_h = attp.tile([128, 16 * T], BF, tag="exp")
                    simv = sim[0:T, :]
                    sim_ap = bass.AP(simv.tensor, simv.offset,
                                     [simv.ap[0], [512, 4], [T, 4], [1, T]])
                    nc.scalar.activation(
                        exp_h[0:T, :].rearrange("p (a s q) -> p a s q", a=4, s=4),
                        sim_ap, AF.Exp)
                    nc.vector.tensor_mul(
                        exp_h[0:T], exp_h[0:T],
                        expb_sb[0:T, hf * 16 * T:(hf + 1) * 16 * T])
                    out2 = attps.tile([128, 2048], F32, tag="ap")
                    for hh in range(16):
                        h = hf * 16 + hh
                        j = _blk(hh)
                        oc = 512 * (hh // 8) + 33 * (hh % 8)
                        nc.tensor.matmul(
                            out2[0:T, oc:oc + 33],
                            lhsT=exp_h[0:T, j * T:(j + 1) * T],
                            rhs=v_b[0:T, h * 33:(h + 1) * 33],
                            start=True, stop=True)
                    rs = atts.tile([128, 16], F32, tag="rs")
                    sv = out2[0:T, 32:33]
                    nc.vector.reciprocal(
                        rs[0:T, :].rearrange("p (a h) -> p a h", a=2),
                        bass.AP(sv.tensor, sv.offset, [sv.ap[0], [512, 2], [33, 8]]))
                    a_sb = attp.tile([128, 512], BF, tag="asb")
                    o2 = out2[0:T, :]
                    in0 = bass.AP(o2.tensor, o2.offset,
                                  [o2.ap[0], [512, 2], [33, 8], [1, 32]])
                    rsv = rs[0:T, :].rearrange("p (a h) -> p a h", a=2)
                    nc.vector.tensor_tensor(
                        a_sb[0:T, :].rearrange("p (a h d) -> p a h d", a=2, h=8),
                        in0, _bc(rsv, 32), ALU.mult)
                    nc.sync.dma_start(
                        out=attn_d[c0:c1, hf * 512:(hf + 1) * 512], in_=a_sb[0:T])

        # ---- phases 7-8: attn_outT + out projection -------------------------
        with tc.tile_pool(name="aT", bufs=1) as aTp, \
             tc.tile_pool(name="oev", bufs=3) as oev, \
             tc.tile_pool(name="ops", bufs=4, space="PSUM") as ops:
            aT = []
            for c in range(8):
                aT_c = aTp.tile([128, NT], BF, tag=f"aT{c}")
                eng = nc.sync  # transposes must share one HWDGE ring:
                # concurrent xbar transposes on both rings corrupt each other
                eng.dma_start_transpose(out=aT_c, in_=attn_d[:, c * 128:(c + 1) * 128])
                aT.append(aT_c)
            for t in range(n_tok_tiles):
                r0, r1 = t * 128, min(t * 128 + 128, NT)
                R = r1 - r0
                ps = [ops.tile([128, 512], F32, tag="op", name=f"op{i}") for i in range(2)]
                for k in range(8):
                    for n in range(2):
                        nc.tensor.matmul(
                            ps[n][:R], lhsT=aT[k][:, r0:r1],
                            rhs=wout_sb[:, k, n * 512:(n + 1) * 512],
                            start=(k == 0), stop=(k == 7))
                o_t = oev.tile([128, DIM], F32, tag="ot")
                for n in range(2):
                    nc.scalar.copy(o_t[:R, n * 512:(n + 1) * 512], ps[n][:R])
                nc.sync.dma_start(out=out_flat[r0:r1], in_=o_t[:R])

    nc.finalize()
    return nc


def build_program2(B, phase_limit=50):
    """v2: fully on-chip, group-pipelined, feature-major program.

    Per core (B windows, T=65): process in groups of WG=8 windows
    (GT=520 tokens).  x arrives bf16 and is DMA-transposed straight into
    feature-major chunks; LN stats come from all-ones PE matmuls (mean and
    E[x^2] replicated across partitions); LN+FiLM applied in-place (DVE);
    q/k are computed transposed (lhsT = w as stored) so they land
    feature-major on-chip (no DRAM round trip); per-head RMS norms come
    from block-diagonal-ones PE matmuls (k's carries 32*gq*gk folded in);
    q is normalized directly into a 2-head block-diagonal layout so sim
    needs only 8 PE instructions per window-half pair; v goes token-major
    through DRAM (repartitioning); attention per window as v1 (simT, exp *
    exp(bias), attnV with ones column, per-partition normalize); attn-out
    is transposed back to feature-major by PE (no DRAM round trip); out
    projection per group.  All phases pipeline across groups via
    double-buffered tiles; PSUM budget: sim 4 banks + out2 2 + shared 2.
    """
    assert B % 8 == 0
    WG = 8
    NG = B // WG
    GT = WG * T            # 520
    NT = B * T
    NTP = NT + 16          # xbf padded so 528-row transposes stay in bounds

    nc = bacc.Bacc("TRN2", target_bir_lowering=False, debug=False)

    xbf = nc.dram_tensor("xbf", [NTP, DIM], BF, kind="ExternalInput").ap()
    condT = nc.dram_tensor("condT", [DIM, B], BF, kind="ExternalInput").ap()
    fw1 = nc.dram_tensor("fw1", [DIM, 2 * DIM], BF, kind="ExternalInput").ap()
    fb1 = nc.dram_tensor("fb1", [2 * DIM], F32, kind="ExternalInput").ap()
    fw2 = nc.dram_tensor("fw2", [2 * DIM, 2 * DIM], BF, kind="ExternalInput").ap()
    fb2 = nc.dram_tensor("fb2", [2 * DIM], F32, kind="ExternalInput").ap()
    wqk = nc.dram_tensor("wqk", [DIM, 2 * DIM], BF, kind="ExternalInput").ap()
    wv = nc.dram_tensor("wv", [DIM, DIM], BF, kind="ExternalInput").ap()
    wout = nc.dram_tensor("wout", [DIM, DIM], BF, kind="ExternalInput").ap()
    onesln = nc.dram_tensor("onesln", [128, 128], BF, kind="ExternalInput").ap()
    bdq = nc.dram_tensor("bdq", [128, 128], BF, kind="ExternalInput").ap()
    bdk = nc.dram_tensor("bdk", [128, 8 * 128], BF, kind="ExternalInput").ap()
    identw = nc.dram_tensor("identw", [128, 128], BF, kind="ExternalInput").ap()
    expbT = nc.dram_tensor("expbT", [T, 2 * 16 * T], BF, kind="ExternalInput").ap()
    out_d = nc.dram_tensor("out", [B, T, DIM], F32, kind="ExternalOutput").ap()

    v_d = nc.dram_tensor("v_d", [NT, HEADS * 33], BF).ap()
    out_flat = out_d.rearrange("b t d -> (b t) d")

    with tile.TileContext(nc) as tc, ExitStack() as top:
        consts = top.enter_context(tc.tile_pool(name="consts", bufs=1))

        wqk_sb = consts.tile([128, 8, 2 * DIM], BF)
        nc.sync.dma_start(out=wqk_sb, in_=wqk.rearrange("(c p) j -> p c j", p=128))
        wv_sb = consts.tile([128, 8, DIM], BF)
        nc.sync.dma_start(out=wv_sb, in_=wv.rearrange("(c p) j -> p c j", p=128))
        wout_sb = consts.tile([128, 8, DIM], BF)
        nc.sync.dma_start(out=wout_sb, in_=wout.rearrange("(c p) j -> p c j", p=128))
        onesln_sb = consts.tile([128, 128], BF)
        nc.sync.dma_start(out=onesln_sb, in_=onesln)
        bdq_sb = consts.tile([128, 128], BF)
        nc.sync.dma_start(out=bdq_sb, in_=bdq)
        bdk_sb = consts.tile([128, 8, 128], BF)
        nc.sync.dma_start(out=bdk_sb, in_=bdk.rearrange("p (c j) -> p c j", j=128))
        ident_sb = consts.tile([128, 128], BF)
        nc.sync.dma_start(out=ident_sb, in_=identw)
        identf_sb = consts.tile([128, 128], F32)
        nc.vector.tensor_copy(identf_sb, ident_sb)
        expb_sb = consts.tile([128, 2 * 16 * T], BF)
        nc.sync.dma_start(out=expb_sb[:T, :], in_=expbT)
        gbT_sb = consts.tile([128, 16, B], BF)
        eps_ln = consts.tile([128, 1], F32)
        nc.vector.memset(eps_ln, 1e-5)
        eps_rms = consts.tile([128, 1], F32)
        nc.vector.memset(eps_rms, 1e-24)

        # ---- FiLM (same as v1) ---------------------------------------------
        with tc.tile_pool(name="film", bufs=1) as filmp, \
             tc.tile_pool(name="filmps", bufs=4, space="PSUM") as filmps:
            fw1_sb = filmp.tile([128, 8, 2 * DIM], BF)
            nc.sync.dma_start(out=fw1_sb, in_=fw1.rearrange("(c p) j -> p c j", p=128))
            fw2_sb = filmp.tile([128, 16, 2 * DIM], BF)
            nc.sync.dma_start(out=fw2_sb, in_=fw2.rearrange("(c p) j -> p c j", p=128))
            fb1_sb = filmp.tile([128, 16], F32)
            nc.sync.dma_start(out=fb1_sb, in_=fb1.rearrange("(c p) -> p c", p=128))
            fb2_sb = filmp.tile([128, 16], F32)
            nc.sync.dma_start(out=fb2_sb, in_=fb2.rearrange("(c p) -> p c", p=128))
            condT_sb = filmp.tile([128, 8, B], BF)
            nc.sync.dma_start(out=condT_sb, in_=condT.rearrange("(c p) b -> p c b", p=128))
            hT_sb = filmp.tile([128, 16, B], BF)

            for c in range(16):
                ps = filmps.tile([128, B], F32, tag="fps")
                for k in range(8):
                    nc.tensor.matmul(
                        ps, lhsT=fw1_sb[:, k, c * 128:(c + 1) * 128],
                        rhs=condT_sb[:, k, :], start=(k == 0), stop=(k == 7))
                lin = filmp.tile([128, B], F32, tag="lin", name=f"lin{c}")
                nc.scalar.activation(lin, ps, AF.Identity,
                                     bias=fb1_sb[:, c:c + 1], scale=1.0)
                sg = filmp.tile([128, B], F32, tag="sg", name=f"sg{c}")
                nc.scalar.activation(sg, ps, AF.Sigmoid,
                                     bias=fb1_sb[:, c:c + 1], scale=1.0)
                nc.vector.tensor_mul(hT_sb[:, c, :], lin, sg)
            for c in range(16):
                ps = filmps.tile([128, B], F32, tag="fps")
                for k in range(16):
                    nc.tensor.matmul(
                        ps, lhsT=fw2_sb[:, k, c * 128:(c + 1) * 128],
                        rhs=hT_sb[:, k, :], start=(k == 0), stop=(k == 15))
                nc.scalar.activation(gbT_sb[:, c, :], ps, AF.Identity,
                                     bias=fb2_sb[:, c:c + 1], scale=1.0)

        # ---- groups ---------------------------------------------------------
        with tc.tile_pool(name="gx", bufs=2) as gxp, \
             tc.tile_pool(name="gkT", bufs=2) as gkp, \
             tc.tile_pool(name="gaT", bufs=2) as gap, \
             tc.tile_pool(name="gwork", bufs=2) as gw, \
             tc.tile_pool(name="gst", bufs=3) as gst, \
             tc.tile_pool(name="ps1", bufs=3, space="PSUM") as ps1, \
             tc.tile_pool(name="ps2", bufs=1, space="PSUM") as ps2, \
             tc.tile_pool(name="psbig", bufs=2, space="PSUM") as psbig, \
             tc.tile_pool(name="psout2", bufs=2, space="PSUM") as psout2:

            for g in range(NG):
                g0 = g * GT
                # transpose-load x chunks (feature-major)
                xf = []
                for c in range(8):
                    xf_c = gxp.tile([128, 528], BF, tag=f"xf{c}", name=f"xf{g}_{c}")
                    nc.sync.dma_start_transpose(
                        out=xf_c, in_=xbf[g0:g0 + 528, c * 128:(c + 1) * 128])
                    xf.append(xf_c)

                # LN stats + apply + FiLM, per 260-token tile (4 windows).
                # rstd = exp(-0.5*ln(var+eps)) keeps ACT in the ln/exp table
                # set (no table thrash with attention's exp); mean/rstd are
                # staged to bf16 so the 16 in-place apply ops run in DVE
                # 16-bit mode.
                for nt in range(2):
                    n0, n1 = nt * 260, (nt + 1) * 260
                    psx = ps1.tile([128, 260], F32, tag="b1", name=f"psx{g}_{nt}")
                    for c in range(8):
                        nc.tensor.matmul(psx, lhsT=onesln_sb, rhs=xf[c][:, n0:n1],
                                         start=(c == 0), stop=(c == 7))
                    psx2 = ps1.tile([128, 260], F32, tag="b1", name=f"psx2{g}_{nt}")
                    for c in range(8):
                        x2 = gw.tile([128, 260], BF, tag="x2")
                        nc.vector.tensor_mul(x2, xf[c][:, n0:n1], xf[c][:, n0:n1])
                        nc.tensor.matmul(psx2, lhsT=onesln_sb, rhs=x2,
                                         start=(c == 0), stop=(c == 7))
                    var = gst.tile([128, 260], F32, tag="var")
                    nc.scalar.activation(var, psx, AF.Square)
                    nc.vector.tensor_tensor(var, psx2, var, ALU.subtract)
                    lnv = gst.tile([128, 260], F32, tag="lnv")
                    nc.scalar.activation(lnv, var, AF.Ln, bias=eps_ln)
                    rstd = gst.tile([128, 260], BF, tag="rstd", name=f"rstd{g}_{nt}")
                    nc.scalar.activation(rstd, lnv, AF.Exp, scale=-0.5)
                    mu = gst.tile([128, 260], BF, tag="mu", name=f"mu{g}_{nt}")
                    nc.vector.tensor_copy(mu, psx)
                    for c in range(8):
                        xv = xf[c][:, n0:n1]
                        nc.vector.tensor_tensor(xv, xv, mu, ALU.subtract)
                        nc.vector.tensor_tensor(xv, xv, rstd, ALU.mult)
                        xw = xf[c][:, n0:n1].rearrange("p (w t) -> p w t", t=T)
                        gcol = g * WG + nt * 4
                        nc.vector.tensor_tensor(
                            xw, xw, _bc(gbT_sb[:, c, gcol:gcol + 4], T), ALU.mult)
                        nc.vector.tensor_tensor(
                            xw, xw, _bc(gbT_sb[:, 8 + c, gcol:gcol + 4], T), ALU.add)

                # V projection (token-major) -> v_d
                for tt in range(5):
                    r0 = tt * 104
                    v_t = gw.tile([128, HEADS, 33], BF, tag="vt")
                    nc.vector.memset(v_t[:, :, 32:33], 1.0)
                    for n2 in range(2):
                        psv = ps1.tile([128, 512], F32, tag="b1", name=f"psv{g}_{tt}_{n2}")
                        for k in range(8):
                            nc.tensor.matmul(
                                psv[:104], lhsT=xf[k][:, r0:r0 + 104],
                                rhs=wv_sb[:, k, n2 * 512:(n2 + 1) * 512],
                                start=(k == 0), stop=(k == 7))
                        nc.scalar.copy(
                            v_t[:104, n2 * 16:(n2 + 1) * 16, 0:32], psv[:104])
                    nc.sync.dma_start(out=v_d[g0 + r0:g0 + r0 + 104], in_=v_t[:104])

                if phase_limit < 2:
                    dum = gw.tile([128, DIM], F32, tag="ot")
                    nc.vector.memset(dum, 0.0)
                    for tt in range(5):
                        r0 = tt * 104
                        nc.sync.dma_start(out=out_flat[g0 + r0:g0 + r0 + 104],
                                          in_=dum[:104])
                    continue
                # q/k transposed projections + RMS norms (feature-major;
                # block-diag-ones matmul replicates per-head sum-of-squares
                # across the head's partitions, k's carries 32*gq*gk folded;
                # rstd = exp(-0.5*ln(s)) -- same ACT table set as exp)
                qT, kT = [], []
                for c in range(8):
                    qT.append(gkp.tile([128, GT], BF, tag=f"qT{c}", name=f"qT{g}_{c}"))
                    kT.append(gkp.tile([128, GT], BF, tag=f"kT{c}", name=f"kT{g}_{c}"))
                for nt in range(2):
                    n0, n1 = nt * 260, (nt + 1) * 260
                    for j in range(16):
                        isq = j < 8
                        c = j if isq else j - 8
                        psqk = ps1.tile([128, 260], F32, tag="b1", name=f"psqk{g}_{nt}_{j}")
                        for k in range(8):
                            nc.tensor.matmul(
                                psqk, lhsT=wqk_sb[:, k, j * 128:(j + 1) * 128],
                                rhs=xf[k][:, n0:n1], start=(k == 0), stop=(k == 7))
                        q2 = gw.tile([128, 260], BF, tag="q2")
                        nc.scalar.activation(q2, psqk, AF.Square)
                        psr = ps2.tile([128, 260], F32, tag="b2", name=f"psr{g}_{nt}_{j}")
                        nc.tensor.matmul(
                            psr, lhsT=(bdq_sb if isq else bdk_sb[:, c, :]), rhs=q2,
                            start=True, stop=True)
                        lns = gst.tile([128, 260], F32, tag="lns")
                        nc.scalar.activation(lns, psr, AF.Ln, bias=eps_rms)
                        rst = gst.tile([128, 260], F32, tag="rst")
                        nc.scalar.activation(rst, lns, AF.Exp, scale=-0.5)
                        dst = (qT if isq else kT)[c][:, n0:n1]
                        nc.vector.tensor_tensor(dst, psqk, rst, ALU.mult)

                # attention per window
                if phase_limit < 21:
                    dum = gw.tile([128, DIM], F32, tag="ot")
                    nc.vector.memset(dum, 0.0)
                    for tt in range(5):
                        r0 = tt * 104
                        nc.sync.dma_start(out=out_flat[g0 + r0:g0 + r0 + 104],
                                          in_=dum[:104])
                    continue
                aT = gap.tile([128, 8, GT], BF, tag="aT", name=f"aT{g}")
                for bw in range(WG):
                    w0 = bw * T
                    v_b = gw.tile([128, HEADS * 33], BF, tag="vb")
                    nc.sync.dma_start(out=v_b[:T], in_=v_d[g0 + w0:g0 + w0 + T])
                    for hf in range(2):
                        exp_h = gw.tile([128, 16 * T], BF, tag="exp")
                        a_sb = gw.tile([128, 512], F32, tag="asb")
                        # sim per rb-group: a PSUM bank may only receive
                        # matmul outputs from ONE PE array row-position, so
                        # group the 4 heads sharing rb = 32*rr into one
                        # 1-bank tile (slots of 65).  Head h = 16hf + rr + 4s
                        # lands at exp block 4*rr + s = _blk(head-in-half).
                        for rr in range(4):
                            sim = psbig.tile([128, 512], F32, tag="big",
                                             name=f"sim{g}_{bw}_{hf}_{rr}")
                            rb = 32 * rr
                            for s in (range(4) if phase_limit >= 22 else []):
                                h = hf * 16 + rr + 4 * s
                                ch = h // 4
                                nc.tensor.matmul(
                                    sim[0:T, T * s:T * s + T],
                                    lhsT=kT[ch][rb:rb + 32, w0:w0 + T],
                                    rhs=qT[ch][rb:rb + 32, w0:w0 + T],
                                    start=True, stop=True,
                                    tile_position=(rb, 0))
                            if phase_limit >= 23:
                                nc.scalar.activation(
                                    exp_h[0:T, 4 * T * rr:4 * T * (rr + 1)].rearrange(
                                        "p (s q) -> p s q", s=4),
                                    sim[0:T, 0:4 * T].rearrange(
                                        "p (s q) -> p s q", s=4),
                                    AF.Exp)
                            if phase_limit >= 24:
                                nc.vector.tensor_mul(
                                    exp_h[0:T, 4 * T * rr:4 * T * (rr + 1)],
                                    exp_h[0:T, 4 * T * rr:4 * T * (rr + 1)],
                                    expb_sb[0:T, (16 * hf + 4 * rr) * T:
                                            (16 * hf + 4 * rr + 4) * T])
                        for hb in range(2):  # attnV in blocks of 8 heads
                            out2 = psout2.tile([128, 512], F32, tag="o2",
                                               name=f"o2{g}_{bw}_{hf}_{hb}")
                            for l in (range(8) if phase_limit >= 25 else []):
                                ih = hb * 8 + l
                                pi = _blk(ih)
                                h = hf * 16 + ih
                                nc.tensor.matmul(
                                    out2[0:T, 33 * l:33 * l + 33],
                                    lhsT=exp_h[0:T, pi * T:(pi + 1) * T],
                                    rhs=v_b[0:T, h * 33:(h + 1) * 33],
                                    start=True, stop=True)
                            if phase_limit < 26:
                                continue
                            rs = gst.tile([128, 8], F32, tag="rs")
                            sv = out2[0:T, 32:33]
                            nc.vector.reciprocal(
                                rs[0:T, :],
                                bass.AP(sv.tensor, sv.offset, [sv.ap[0], [33, 8]]))
                            o2 = out2[0:T, :]
                            in0 = bass.AP(o2.tensor, o2.offset,
                                          [o2.ap[0], [33, 8], [1, 32]])
                            nc.vector.tensor_tensor(
                                a_sb[0:T, 256 * hb:256 * (hb + 1)].rearrange(
                                    "p (h d) -> p h d", h=8),
                                in0, _bc(rs[0:T, :], 32), ALU.mult)
                        if phase_limit >= 40:
                            tps = ps2.tile([128, 4, T], F32, tag="b2",
                                           name=f"tps{g}_{bw}_{hf}")
                            for q4 in range(4):
                                nc.tensor.transpose(
                                    tps[:, q4, 0:T],
                                    a_sb[0:T, q4 * 128:(q4 + 1) * 128],
                                    identf_sb[0:T, 0:T])
                            dst = aT[:, hf * 4:hf * 4 + 4, w0:w0 + T]
                            nc.scalar.copy(dst, tps[:, :, 0:T])

                # out projection
                if phase_limit < 50:
                    dum = gw.tile([128, DIM], F32, tag="ot")
                    nc.vector.memset(dum, 0.0)
                    for tt in range(5):
                        r0 = tt * 104
                        nc.sync.dma_start(out=out_flat[g0 + r0:g0 + r0 + 104],
                                          in_=dum[:104])
                    continue
                for tt in range(5):
                    r0 = tt * 104
                    o_t = gw.tile([128, DIM], F32, tag="ot")
                    for n2 in range(2):
                        pso = ps1.tile([128, 512], F32, tag="b1", name=f"pso{g}_{tt}_{n2}")
                        for k in range(8):
                            nc.tensor.matmul(
                                pso[:104], lhsT=aT[:, k, r0:r0 + 104],
                                rhs=wout_sb[:, k, n2 * 512:(n2 + 1) * 512],
                                start=(k == 0), stop=(k == 7))
                        nc.vector.tensor_copy(o_t[:104, n2 * 512:(n2 + 1) * 512],
                                              pso[:104])
                    nc.sync.dma_start(out=out_flat[g0 + r0:g0 + r0 + 104],
                                      in_=o_t[:104])

    nc.finalize()
    return nc


def host_inputs2(x, cond, film_w1, film_b1, film_w2, film_b2, w_qkv,
                 q_gamma, k_gamma, rel_bias_table, w_out, n_cores=N_CORES):
    """Per-core input maps for build_program2."""
    Bfull = x.shape[0]
    B = Bfull // n_cores
    NT = B * T
    bias = np.asarray(rel_bias_table, np.float32)[REL_IDX]  # (q=i, k=j, h)
    expb = np.exp(bias.astype(np.float64)).astype(np.float32)
    # expbT block p holds head 16*hf + _blk(p): the sim rb-group layout
    # (block 4*rr + s holds head rr + 4*s, and _blk is self-inverse)
    expbT = np.zeros((T, 2 * 16 * T), np.float32)
    for hf in range(2):
        for p in range(16):
            h = 16 * hf + _blk(p)
            expbT[:, (hf * 16 + p) * T:(hf * 16 + p + 1) * T] = expb[:, :, h].T
    gc = (32.0 * np.asarray(q_gamma, np.float32).reshape(HEADS, DIM_HEAD)
          * np.asarray(k_gamma, np.float32).reshape(HEADS, DIM_HEAD)).reshape(-1)
    assert (gc > 0).all(), "feature-major RMS fold assumes positive gamma product"
    onesln = np.full((128, 128), 1.0 / 1024.0, np.float32)
    blk = np.kron(np.eye(4, dtype=np.float32), np.ones((32, 32), np.float32))
    bdq = blk
    # bdk[:, c, e] = blockdiag / gc[c*128+e]^2  -> rsqrt gives gc/|k|
    bdk = np.zeros((128, 8, 128), np.float32)
    for c in range(8):
        bdk[:, c, :] = blk / (gc[c * 128:(c + 1) * 128][None, :] ** 2)
    ident = np.eye(128, dtype=np.float32)

    xf = np.asarray(x, np.float32).reshape(Bfull * T, DIM)
    shared = {
        "fw1": np.asarray(film_w1).astype(BF16),
        "fb1": np.asarray(film_b1, np.float32),
        "fw2": np.asarray(film_w2).astype(BF16),
        "fb2": np.asarray(film_b2, np.float32),
        "wqk": np.asarray(w_qkv[:, :2 * DIM]).astype(BF16),
        "wv": np.ascontiguousarray(w_qkv[:, 2 * DIM:]).astype(BF16),
        "wout": np.asarray(w_out).astype(BF16),
        "onesln": onesln.astype(BF16),
        "bdq": bdq.astype(BF16),
        "bdk": bdk.reshape(128, 8 * 128).astype(BF16),
        "identw": ident.astype(BF16),
        "expbT": expbT.astype(BF16),
    }
    in_maps = []
    for i in range(n_cores):
        m = dict(shared)
        xc = xf[i * NT:(i + 1) * NT].astype(BF16)
        xp = np.zeros((NT + 16, DIM), BF16)
        xp[:NT] = xc
        m["xbf"] = xp
        m["condT"] = np.ascontiguousarray(
            np.asarray(cond[i * B:(i + 1) * B], np.float32).T).astype(BF16)
        in_maps.append(m)
    return in_maps


def host_inputs(x, cond, film_w1, film_b1, film_w2, film_b2, w_qkv,
                q_gamma, k_gamma, rel_bias_table, w_out, n_cores=N_CORES):
    """Build the per-core input maps (host-side staging only)."""
    Bfull = x.shape[0]
    B = Bfull // n_cores
    bias = np.asarray(rel_bias_table, np.float32)[REL_IDX]  # (q, k, h)
    expb = np.exp(bias.astype(np.float64)).astype(np.float32)
    expbT = np.zeros((T, 2 * 16 * T), np.float32)
    for hf in range(2):
        for j in range(16):
            H = hf * 16 + _blk(j)
            expbT[:, (hf * 16 + j) * T:(hf * 16 + j + 1) * T] = expb[:, :, H].T
    gcomb = (32.0 * np.asarray(q_gamma, np.float32).reshape(HEADS, DIM_HEAD)
             * np.asarray(k_gamma, np.float32).reshape(HEADS, DIM_HEAD)).reshape(-1)
    shared = {
        "fw1": np.asarray(film_w1).astype(BF16),
        "fb1": np.asarray(film_b1, np.float32),
        "fw2": np.asarray(film_w2).astype(BF16),
        "fb2": np.asarray(film_b2, np.float32),
        "wqk": np.asarray(w_qkv[:, :2 * DIM]).astype(BF16),
        "wv": np.ascontiguousarray(w_qkv[:, 2 * DIM:]).astype(BF16),
        "wout": np.asarray(w_out).astype(BF16),
        "gcomb": gcomb.astype(np.float32),
        "expbT": expbT.astype(BF16),
    }
    in_maps = []
    for i in range(n_cores):
        m = dict(shared)
        m["x"] = np.ascontiguousarray(x[i * B:(i + 1) * B], np.float32)
        m["condT"] = np.ascontiguousarray(
            np.asarray(cond[i * B:(i + 1) * B], np.float32).T).astype(BF16)
        in_maps.append(m)
    return in_maps


_PROGRAM_CACHE = {}

# v2: group-pipelined on-chip program.  Key HW constraint found by bisection:
# a PSUM bank may only receive matmul outputs from ONE PE-array row position
# (tile_position), so sim groups the 4 heads sharing rb=32*rr per bank.
VERSION = 2


def _get_program(B):
    key = (VERSION, B)
    if key not in _PROGRAM_CACHE:
        _PROGRAM_CACHE[key] = (build_program2(B) if VERSION == 2
                               else build_program(B))
    return _PROGRAM_CACHE[key]


def _host_inputs(**inputs):
    return host_inputs2(**inputs) if VERSION == 2 else host_inputs(**inputs)


def run(inputs, trace=False, tmpdir=None):
    from concourse.bass_utils import run_bass_kernel_spmd

    x = np.asarray(inputs["x"], np.float32)
    B = x.shape[0] // N_CORES
    nc = _get_program(B)
    in_maps = _host_inputs(**inputs)
    res = run_bass_kernel_spmd(nc, in_maps, core_ids=list(range(N_CORES)),
                               trace=trace, tmpdir=tmpdir)
    out = np.concatenate([np.asarray(r["out"]) for r in res.results], axis=0)
    return out.astype(np.float32), res


def kernel(x, cond, film_w1, film_b1, film_w2, film_b2, w_qkv,
           q_gamma, k_gamma, rel_bias_table, w_out):
    out, _ = run(dict(
        x=x, cond=cond, film_w1=film_w1, film_b1=film_b1, film_w2=film_w2,
        film_b2=film_b2, w_qkv=w_qkv, q_gamma=q_gamma, k_gamma=k_gamma,
        rel_bias_table=rel_bias_table, w_out=w_out))
    return out


def run_slope(inputs, n_lo=8, n_hi=72, rounds=9):
    """Measure per-execution device time by async-queuing chains of
    sequentially-dependent executions (output buffers threaded through
    donation) and taking the slope between an n_hi-chain and an n_lo-chain
    wall time: slope = (T_hi - T_lo) / (n_hi - n_lo).  The fixed dispatch /
    relay overhead (~90ms here) cancels; each link in the chain is a full
    kernel execution (all HBM traffic + compute), serialized by the data
    dependency.  Returns (out_full, per_exec_seconds_median, slopes)."""
    import time as _time
    import jax
    import numpy as _np
    from jax.sharding import Mesh, PartitionSpec, NamedSharding
    from jax.experimental.shard_map import shard_map
    from concourse import bass2jax, mybir as _mybir

    bass2jax.install_neuronx_cc_hook()
    x = np.asarray(inputs["x"], np.float32)
    B = x.shape[0] // N_CORES
    nc = _get_program(B)
    in_maps = _host_inputs(**inputs)

    in_names, out_names, out_avals, zero_shapes = [], [], [], []
    for alloc in nc.m.functions[0].allocations:
        if not isinstance(alloc, _mybir.MemoryLocationSet):
            continue
        name = alloc.memorylocations[0].name
        if alloc.kind == "ExternalInput":
            if nc.partition_id_tensor is None or name != nc.partition_id_tensor.name:
                in_names.append(name)
        elif alloc.kind == "ExternalOutput":
            out_names.append(name)
            shape = tuple(alloc.tensor_shape)
            dtype = _mybir.dt.np(alloc.dtype)
            out_avals.append(jax.core.ShapedArray(shape, dtype))
            zero_shapes.append((shape, dtype))
    n_params = len(in_names)
    all_in_names = in_names + out_names
    if nc.partition_id_tensor is not None:
        all_in_names = all_in_names + [nc.partition_id_tensor.name]
    oi = out_names.index("out")

    def _body(*args):
        operands = list(args)
        if nc.partition_id_tensor is not None:
            operands.append(bass2jax.partition_id_tensor())
        outs = bass2jax._bass_exec_p.bind(
            *operands, out_avals=tuple(out_avals), in_names=tuple(all_in_names),
            out_names=tuple(out_names), lowering_input_output_aliases=(),
            sim_require_finite=True, sim_require_nnan=True, nc=nc)
        return tuple(outs)

    devices = jax.devices()[:N_CORES]
    mesh = Mesh(_np.asarray(devices), ("core",))
    n_outs = len(out_names)
    donate = tuple(range(n_params, n_params + n_outs))
    shard = NamedSharding(mesh, PartitionSpec("core"))
    fn = jax.jit(
        shard_map(_body, mesh=mesh,
                  in_specs=(PartitionSpec("core"),) * (n_params + n_outs),
                  out_specs=(PartitionSpec("core"),) * n_outs,
                  check_rep=False),
        donate_argnums=donate, keep_unused=True)

    dev_in = [
        jax.device_put(_np.concatenate(
            [_np.asarray(in_maps[c][name]) for c in range(N_CORES)], axis=0), shard)
        for name in in_names
    ]
    jax.block_until_ready(dev_in)

    outs = []
    for s, d in zero_shapes:
        f = jax.jit(lambda s=s, d=d: jax.numpy.zeros((N_CORES * s[0], *s[1:]), d),
                    out_shardings=shard)
        outs.append(f())
    outs = fn(*dev_in, *outs)  # warm-up (compiles the wrapper)
    jax.block_until_ready(outs)

    slopes = []
    for _ in range(rounds):
        ts = {}
        for n in (n_lo, n_hi):
            t0 = _time.perf_counter()
            for _i in range(n):
                outs = fn(*dev_in, *outs)
            jax.block_until_ready(outs)
            ts[n] = _time.perf_counter() - t0
        slopes.append((ts[n_hi] - ts[n_lo]) / (n_hi - n_lo))
    slopes.sort()
    # relay/device interference only ever adds time, so the upper slopes are
    # contaminated; take the median of the lower half as the steady estimate
    lower = slopes[:max(1, (len(slopes) + 1) // 2)]
    per_exec = lower[len(lower) // 2]

    full = _np.asarray(outs[oi]).reshape(N_CORES * B, T, DIM).astype(_np.float32)
    return full, per_exec, slopes


def run_timed(inputs, iters=10):
    """Execute on 8 cores with device-resident inputs; time execute-only.

    Returns (out_full, per_iter_seconds). Mirrors bass2jax.run_bass_via_pjrt
    but pre-places inputs on the device mesh so the timed region covers only
    the sharded NEFF execution (plus PJRT dispatch).
    """
    import jax
    import numpy as _np
    from jax.sharding import Mesh, PartitionSpec, NamedSharding
    from jax.experimental.shard_map import shard_map
    from concourse import bass2jax, mybir as _mybir
    import time as _time

    bass2jax.install_neuronx_cc_hook()
    x = np.asarray(inputs["x"], np.float32)
    B = x.shape[0] // N_CORES
    nc = _get_program(B)
    in_maps = _host_inputs(**inputs)

    in_names, out_names, out_avals, zero_shapes = [], [], [], []
    for alloc in nc.m.functions[0].allocations:
        if not isinstance(alloc, _mybir.MemoryLocationSet):
            continue
        name = alloc.memorylocations[0].name
        if alloc.kind == "ExternalInput":
            if nc.partition_id_tensor is None or name != nc.partition_id_tensor.name:
                in_names.append(name)
        elif alloc.kind == "ExternalOutput":
            out_names.append(name)
            shape = tuple(alloc.tensor_shape)
            dtype = _mybir.dt.np(alloc.dtype)
            out_avals.append(jax.core.ShapedArray(shape, dtype))
            zero_shapes.append((shape, dtype))
    n_params = len(in_names)
    all_in_names = in_names + out_names
    if nc.partition_id_tensor is not None:
        all_in_names = all_in_names + [nc.partition_id_tensor.name]

    def _body(*args):
        operands = list(args)
        if nc.partition_id_tensor is not None:
            operands.append(bass2jax.partition_id_tensor())
        outs = bass2jax._bass_exec_p.bind(
            *operands,
            out_avals=tuple(out_avals),
            in_names=tuple(all_in_names),
            out_names=tuple(out_names),
            lowering_input_output_aliases=(),
            sim_require_finite=True,
            sim_require_nnan=True,
            nc=nc,
        )
        return tuple(outs)

    devices = jax.devices()[:N_CORES]
    mesh = Mesh(_np.asarray(devices), ("core",))
    n_outs = len(out_names)
    donate = tuple(range(n_params, n_params + n_outs))
    sharded = jax.jit(
        shard_map(_body, mesh=mesh,
                  in_specs=(PartitionSpec("core"),) * (n_params + n_outs),
                  out_specs=(PartitionSpec("core"),) * n_outs,
                  check_rep=False),
        donate_argnums=donate, keep_unused=True)

    shard = NamedSharding(mesh, PartitionSpec("core"))
    dev_in = [
        jax.device_put(_np.concatenate(
            [_np.asarray(in_maps[c][name]) for c in range(N_CORES)], axis=0), shard)
        for name in in_names
    ]
    def fresh_zeros():
        return [jax.device_put(
            _np.zeros((N_CORES * s[0], *s[1:]), d), shard) for s, d in zero_shapes]

    # warm-up (compiles)
    outs = sharded(*dev_in, *fresh_zeros())
    jax.block_until_ready(outs)

    zsets = [fresh_zeros() for _ in range(iters)]
    times = []
    for z in zsets:
        t0 = _time.perf_counter()
        outs = sharded(*dev_in, *z)
        jax.block_until_ready(outs)
        times.append(_time.perf_counter() - t0)

    oi = out_names.index("out")
    full = _np.asarray(outs[oi]).reshape(N_CORES * B, T, DIM).astype(_np.float32)
    return full, times


def run_chained(inputs, n_chain=8, iters=5):
    """Measure per-execution device time by chaining n_chain sequential
    executions of the NEFF inside one PJRT dispatch (the 'out' donation
    buffer threads a data dependency), then comparing against a 1-chain
    dispatch. Returns (out, times_1, times_n)."""
    import jax
    import numpy as _np
    from jax.sharding import Mesh, PartitionSpec, NamedSharding
    from jax.experimental.shard_map import shard_map
    from concourse import bass2jax, mybir as _mybir
    import time as _time

    bass2jax.install_neuronx_cc_hook()
    x = np.asarray(inputs["x"], np.float32)
    B = x.shape[0] // N_CORES
    nc = _get_program(B)
    in_maps = _host_inputs(**inputs)

    in_names, out_names, out_avals, zero_shapes = [], [], [], []
    for alloc in nc.m.functions[0].allocations:
        if not isinstance(alloc, _mybir.MemoryLocationSet):
            continue
        name = alloc.memorylocations[0].name
        if alloc.kind == "ExternalInput":
            if nc.partition_id_tensor is None or name != nc.partition_id_tensor.name:
                in_names.append(name)
        elif alloc.kind == "ExternalOutput":
            out_names.append(name)
            shape = tuple(alloc.tensor_shape)
            dtype = _mybir.dt.np(alloc.dtype)
            out_avals.append(jax.core.ShapedArray(shape, dtype))
            zero_shapes.append((shape, dtype))
    n_params = len(in_names)
    all_in_names = in_names + out_names
    if nc.partition_id_tensor is not None:
        all_in_names = all_in_names + [nc.partition_id_tensor.name]
    oi = out_names.index("out")

    def _exec_once(ins, outbufs):
        operands = list(ins) + list(outbufs)
        if nc.partition_id_tensor is not None:
            operands.append(bass2jax.partition_id_tensor())
        return bass2jax._bass_exec_p.bind(
            *operands, out_avals=tuple(out_avals), in_names=tuple(all_in_names),
            out_names=tuple(out_names), lowering_input_output_aliases=(),
            sim_require_finite=True, sim_require_nnan=True, nc=nc)

    def make_body(n):
        def _body(*args):
            ins = args[:n_params]
            outbufs = list(args[n_params:])
            for _ in range(n):
                outs = _exec_once(ins, outbufs)
                outbufs = list(outs)
            return tuple(outbufs)
        return _body

    devices = jax.devices()[:N_CORES]
    mesh = Mesh(_np.asarray(devices), ("core",))
    n_outs = len(out_names)
    donate = tuple(range(n_params, n_params + n_outs))
    shard = NamedSharding(mesh, PartitionSpec("core"))
    fns = {}
    for n in (1, n_chain):
        fns[n] = jax.jit(
            shard_map(make_body(n), mesh=mesh,
                      in_specs=(PartitionSpec("core"),) * (n_params + n_outs),
                      out_specs=(PartitionSpec("core"),) * n_outs,
                      check_rep=False),
            donate_argnums=donate, keep_unused=True)

    dev_in = [
        jax.device_put(_np.concatenate(
            [_np.asarray(in_maps[c][name]) for c in range(N_CORES)], axis=0), shard)
        for name in in_names
    ]
    def fresh_zeros():
        return [jax.device_put(
            _np.zeros((N_CORES * s[0], *s[1:]), d), shard) for s, d in zero_shapes]

    results = {}
    out_final = None
    for n in (1, n_chain):
        outs = fns[n](*dev_in, *fresh_zeros())
        jax.block_until_ready(outs)
        ts = []
        for _ in range(iters):
            z = fresh_zeros()
            t0 = _time.perf_counter()
            outs = fns[n](*dev_in, *z)
            jax.block_until_ready(outs)
            ts.append(_time.perf_counter() - t0)
        results[n] = ts
        out_final = outs
    full = _np.asarray(out_final[oi]).reshape(N_CORES * B, T, DIM).astype(_np.float32)
    return full, results[1], results[n_chain]

